# revision 25
# baseline (speedup 1.0000x reference)
"""Trainium2 Bass kernel for nn_Conv2d_lsq_int (LSQ int8-style quantized 3x3 conv).

Full-input contract: kernel(**inputs) takes the complete tensors
(x[16,320,64,64], weight[320,320,3,3], bias[320], scalar step sizes) and
returns the full [16,320,64,64] float32 output.

Distribution: data-parallel over the batch dim — 2 images per NeuronCore on
8 cores; weight/bias replicated. The conv itself, input/weight quantization
and the shift/round/clip epilogue all run on device; the host only shards the
batch, re-lays-out the weight to a cout-chunk-major [ci, (cot, k, co)] order
(pure data movement), computes the 320-element bias requant (DVE has no
divide op), and concatenates the per-core outputs.

Math notes:
 - x_int/w_int are integers in [-127,127] -> exact in bf16; the 3x3 conv is
   computed as 9 shifted matmuls per 128-channel cin chunk accumulating fp32
   in PSUM (every intermediate is an exactly-representable integer, so the
   result matches the reference bit-for-bit regardless of order).
 - round() is fp32 add/subtract of 1.5*2^23 (round-to-nearest-even, identical
   to jnp.round), fused as a single-rounding FMA in ScalarE (Copy,scale,bias).
 - cin = 320 = 2.5 * 128: the 64-wide remainder chunk would waste half the PE
   array, so remainder matmuls for the two row-tiles of a pair are packed
   into the two halves of the array (x/w for cin 256:320 duplicated on
   partitions 64:127); the 64-wide cout remainder is column-packed the same
   way. Packed matmul pairs co-issue on the PE tile quadrants.

Schedule notes (v2):
 - Weights are staged and quantized in 9 (cin-chunk x cout-chunk) pieces in
   first-consumption order (the very first piece in 3 column sub-slices), so
   the first matmul's weight slice is ready ~2us in instead of ~15us.
 - x is loaded in row-slabs sized so row-tile pair p of an image depends only
   on slabs <= p (17/16/16/15 rows; the first slab of image 0 split 9+8).
 - Per pair, 6 PSUM banks are opened (A/B row tiles x 3 cout chunks) and
   matmuls are grouped by PE-array shape (full 128x128, then 64-deep cin
   remainders, then 64-wide cout column-packed, then 64x64 quadrant packed)
   to minimize shape-transition stalls; the small 64-wide banks drain last,
   keeping the end-of-kernel tail short.
 - Image 1's slab loads/quant are emitted interleaved between image 0's
   pairs so the per-engine program order matches true consumption order.
"""

import contextlib
import ctypes
import sys
import types

import numpy as np

import concourse.bass as bass  # noqa: F401
import concourse.tile as tile
from concourse import bacc, mybir
from concourse.bass_utils import run_bass_kernel_spmd

F32 = mybir.dt.float32
BF16 = mybir.dt.bfloat16
OP = mybir.AluOpType
ACTF = mybir.ActivationFunctionType

MAGIC = 12582912.0  # 1.5 * 2**23 : fp32 round-to-nearest-even trick
QMAX = 127.0

B, CIN, COUT, H, W, K = 16, 320, 320, 64, 64, 3
N_CORES = 8
IMGS_PER_CORE = B // N_CORES
HW = H * W
PW = W + 2  # padded width
PH = H + 2
ROWS_PER_TILE = 8  # 8 rows * 64 cols = 512 px per psum tile
CHUNKS = [(0, 128), (128, 128), (256, 64)]  # (start, size) along cin / cout
COT_OFF = [0, K * K * 128, K * K * 256]  # col offset of each cout chunk block
# x row-slabs: pair p of an image only needs slabs <= p
SLABS_I0 = [(0, 9), (9, 8), (17, 16), (33, 16), (49, 15)]
SLABS_I1 = [(0, 17), (17, 16), (33, 16), (49, 15)]
QUAD4 = True  # 4-way quadrant packing of the cin+cout remainder


def _install_axon_ntff_hook():
    """Slim antenv.axon_hooks so trace=True works (and never crashes) here."""
    if "antenv.axon_hooks" in sys.modules:
        return
    hook = None
    try:
        lib = ctypes.CDLL("/opt/axon/libaxon_pjrt.so")
        if hasattr(lib, "axon_start_nrt_profile"):
            lib.axon_start_nrt_profile.argtypes = [
                ctypes.POINTER(ctypes.c_int64),
                ctypes.c_size_t,
            ]
            lib.axon_start_nrt_profile.restype = ctypes.c_int64
            lib.axon_stop_nrt_profile.argtypes = [ctypes.c_char_p]
            lib.axon_stop_nrt_profile.restype = ctypes.c_int64

            @contextlib.contextmanager
            def hook(output_dir, device_ids):  # noqa: F811
                import jax

                jax.devices()
                if device_ids:
                    ids = (ctypes.c_int64 * len(device_ids))(*device_ids)
                    rc = lib.axon_start_nrt_profile(ids, len(device_ids))
                else:
                    rc = lib.axon_start_nrt_profile(None, 0)
                if rc != 0:
                    raise RuntimeError(f"axon_start_nrt_profile rc={rc}")
                try:
                    yield
                finally:
                    n = lib.axon_stop_nrt_profile(str(output_dir).encode())
                    print(f"profile: {n} ntff file(s) -> {output_dir}",
                          file=sys.stderr)
    except OSError:
        pass

    mod = types.ModuleType("antenv.axon_hooks")
    mod.get_axon_ntff_profile_hook = lambda: hook
    mod.set_axon_ntff_profile_hook = lambda h: None
    sys.modules["antenv.axon_hooks"] = mod

    # keep profiling artifacts local (zero-egress container)
    import concourse.bass_utils as bu

    bu.upload_artifacts = lambda tmpdir: "local://" + str(tmpdir)


def bias_int8(b, sb, ss, sx, sw):
    """Host fp32 replica of the reference's bias requant (DVE lacks divide).

    Every op is a single IEEE-754 fp32 operation in the reference's exact
    order, so this is bit-identical to the jax fp32 computation.
    """
    f32 = np.float32
    b = np.asarray(b, np.float32)
    b_deq = np.clip(np.round(b / f32(sb)), -QMAX, QMAX).astype(np.float32) * f32(sb)
    x_scale = f32(1.0) / f32(sx)
    w_scale = f32(1.0) / f32(sw)
    t = ((b_deq * f32(ss)) * x_scale) * w_scale
    return np.clip(np.round(t), -QMAX, QMAX).astype(np.float32)


def prep_weight(w):
    """Host layout prep (pure data movement):

    [co, ci, kh, kw] -> [ci, (cout-chunk, k, co-within-chunk)] so each
    (cin-chunk, cout-chunk) weight piece is one contiguous DMA."""
    wt = np.transpose(np.asarray(w, np.float32), (1, 2, 3, 0)).reshape(
        CIN, K * K, COUT
    )
    blocks = [
        np.ascontiguousarray(wt[:, :, co0 : co0 + cs]).reshape(CIN, K * K * cs)
        for co0, cs in CHUNKS
    ]
    return np.ascontiguousarray(np.concatenate(blocks, axis=1))


def _build(sx: float, sw: float, sb: float, ss: float):
    """Build the per-core Bass program. Scalars are baked as immediates."""
    nc = bacc.Bacc("TRN2", target_bir_lowering=False, debug=False)

    x_d = nc.dram_tensor("x", [IMGS_PER_CORE, CIN, HW], F32, kind="ExternalInput")
    w_d = nc.dram_tensor("w", [CIN, K * K * COUT], F32, kind="ExternalInput")
    b_d = nc.dram_tensor("b", [COUT], F32, kind="ExternalInput")
    y_d = nc.dram_tensor("y", [IMGS_PER_CORE, COUT, HW], F32, kind="ExternalOutput")

    r_x = float(np.float32(1.0) / np.float32(sx))  # x_scale
    r_w = float(np.float32(1.0) / np.float32(sw))  # w_scale
    ss_f = float(np.float32(ss))

    with tile.TileContext(nc) as tc:
        with (
            tc.tile_pool(name="persist", bufs=1) as persist,
            tc.tile_pool(name="wstage", bufs=3) as wstage,
            tc.tile_pool(name="xstage", bufs=6) as xstage,
            tc.tile_pool(name="epi", bufs=4) as epi,
            tc.tile_pool(name="psum", bufs=8, space="PSUM") as psum,
        ):
            # --- padded bf16 image buffers; zero only the border ring ------
            # (interior fully overwritten by quant writes). Border memsets on
            # the idle Pool/GpSimd queue so ACT/DVE start quant work at t=0.
            xq = {}

            def make_xq(i):
                for c in range(len(CHUNKS)):
                    xq_t = persist.tile(
                        [128, PH * PW], BF16, tag=f"xq{i}_{c}", name=f"xq{i}_{c}"
                    )
                    x3 = xq_t.rearrange("p (r w) -> p r w", r=PH)
                    eng = nc.gpsimd
                    eng.memset(x3[:, 0:1, :], 0.0)       # top pad row
                    eng.memset(x3[:, PH - 1 :, :], 0.0)  # bottom pad row
                    eng.memset(x3[:, 1 : PH - 1, 0:1], 0.0)       # left col
                    eng.memset(x3[:, 1 : PH - 1, PW - 1 :], 0.0)  # right col
                    xq[(i, c)] = x3

            # ---------------- weights: quantize to bf16 pieces -------------
            # wq[c] column layout: [cout-chunk block][k][co]; cin remainder
            # chunk (c=2) duplicated on partitions 64:127 for packed matmuls.
            wq = [
                persist.tile(
                    [128, K * K * COUT], BF16, tag=f"wq{c}", name=f"wq{c}"
                )
                for c in range(3)
            ]

            def emit_w_piece(c, cot, bounds=None):
                ci0, pc = CHUNKS[c]
                _, cs = CHUNKS[cot]
                off = COT_OFF[cot]
                wcols = K * K * cs
                wst = wstage.tile([128, K * K * 128], F32, tag="wst",
                                  name=f"wst{c}_{cot}")
                if bounds is None:
                    bounds = [(0, wcols)]
                for lo, hi in bounds:
                    nc.sync.dma_start(
                        wst[:pc, lo:hi], w_d[ci0 : ci0 + pc, off + lo : off + hi]
                    )
                for lo, hi in bounds:
                    # round via magic FMA; (sub, min) writes bf16 directly --
                    # values below -127 round coarsely in bf16 but are then
                    # clamped to exactly -127 by the in-place max; the cin
                    # remainder is quantized once on 64 partitions and then
                    # duplicated by a cheap SBUF-to-SBUF DMA (no HBM traffic)
                    nc.scalar.activation(
                        wst[:pc, lo:hi], wst[:pc, lo:hi], ACTF.Copy,
                        bias=MAGIC, scale=r_w,
                    )
                    dst = wq[c][:pc, off + lo : off + hi]
                    nc.vector.tensor_scalar(
                        dst, wst[:pc, lo:hi], MAGIC, QMAX, OP.subtract, OP.min,
                    )
                    nc.vector.tensor_scalar(dst, dst, -QMAX, None, OP.max)
                    if pc < 128:
                        nc.sync.dma_start(
                            wq[c][pc : 2 * pc, off + lo : off + hi], dst
                        )

            def emit_x_slab(i, r0, nrows, only_c=None, dma_eng=None):
                # image-1 slabs DMA from the Scalar hw-DGE queue: the Sync
                # queue is in-order and its y-output descriptors wait on
                # epilogue completion, which would convoy-block these loads
                dma = dma_eng if dma_eng is not None else nc.sync
                for c, (ci0, pc) in enumerate(CHUNKS):
                    if only_c is not None and c != only_c:
                        continue
                    st = xstage.tile([128, 17 * W], F32, tag="xst")
                    dma.dma_start(
                        st[:pc, : nrows * W],
                        x_d[i, ci0 : ci0 + pc, r0 * W : (r0 + nrows) * W],
                    )
                    # round via magic FMA (single-rounding); (sub, min) writes
                    # bf16 into the padded buffer -- values below -127 round
                    # coarsely in bf16 but are then clamped to exactly -127
                    # by the in-place max; the cin remainder is quantized on
                    # 64 partitions and duplicated by SBUF-to-SBUF DMA
                    nc.scalar.activation(
                        st[:pc, : nrows * W], st[:pc, : nrows * W], ACTF.Copy,
                        bias=MAGIC, scale=r_x,
                    )
                    dst = xq[(i, c)][:pc, 1 + r0 : 1 + r0 + nrows, 1 : W + 1]
                    srcv = st[:pc, : nrows * W].rearrange(
                        "p (r w) -> p r w", r=nrows
                    )
                    nc.vector.tensor_scalar(
                        dst, srcv, MAGIC, QMAX, OP.subtract, OP.min
                    )
                    nc.vector.tensor_scalar(dst, dst, -QMAX, None, OP.max)
                    if pc < 128:
                        dma.dma_start(
                            xq[(i, c)][
                                pc : 2 * pc, 1 + r0 : 1 + r0 + nrows, 1 : W + 1
                            ],
                            dst,
                        )

            # ---- startup: interleave w pieces and image-0 slabs in ----
            # first-consumption order (pair-0 pieces first, then the rest)
            make_xq(0)
            emit_x_slab(0, *SLABS_I0[0], only_c=0)
            emit_w_piece(0, 0, bounds=[(0, 128), (128, 640), (640, 1152)])
            emit_x_slab(0, *SLABS_I0[0], only_c=1)
            emit_w_piece(1, 0)
            emit_x_slab(0, *SLABS_I0[1], only_c=0)
            emit_x_slab(0, *SLABS_I0[1], only_c=1)
            emit_w_piece(0, 1)
            emit_w_piece(1, 1)
            emit_x_slab(0, *SLABS_I0[0], only_c=2)
            emit_x_slab(0, *SLABS_I0[1], only_c=2)
            emit_w_piece(2, 0)
            emit_w_piece(2, 1)
            emit_w_piece(0, 2, bounds=[(0, 320), (320, 576)])
            emit_w_piece(1, 2, bounds=[(0, 320), (320, 576)])
            emit_w_piece(2, 2)

            # ------------- b_int8 (host-computed), laid out [128, 4] ------
            # col 3 duplicates col 2 on partitions 64:127 for the
            # column-packed cout-remainder epilogue.
            bt = persist.tile([128, 4], F32, tag="bias")
            nc.vector.memset(bt[:], 0.0)
            nc.sync.dma_start(
                bt[:, 0:2], b_d[0:256].rearrange("(c p) -> p c", p=128)
            )
            nc.sync.dma_start(
                bt[:64, 2:3], b_d[256:320].rearrange("(p c) -> p c", c=1)
            )
            nc.sync.dma_start(
                bt[64:128, 3:4], b_d[256:320].rearrange("(p c) -> p c", c=1)
            )

            emit_x_slab(0, *SLABS_I0[2])
            emit_x_slab(0, *SLABS_I0[3])
            emit_x_slab(0, *SLABS_I0[4])

            # ---------------- main conv loop ------------------------------
            def emit_epilogue(ps, p0, cot, co0, cs, i, r0, nsplit=1,
                              extra=None):
                # p0: psum/base partition of this output block. nsplit>1
                # pipelines the chain in column slices (shorter drain tail).
                # extra: second psum bank to sum in (4-way-packed remainder).
                t1 = epi.tile([128, ROWS_PER_TILE * W], F32, tag="t1")
                t2 = epi.tile([128, ROWS_PER_TILE * W], F32, tag="t2")
                bcol = cot if p0 == 0 else 3
                nw = ROWS_PER_TILE * W
                for j in range(nsplit):
                    lo, hi = nw * j // nsplit, nw * (j + 1) // nsplit
                    if extra is not None:
                        # sum the two banks pre-round: ss is a power of two,
                        # so both products and their sum are exact in fp32;
                        # only one PSUM operand per DVE op is allowed
                        nc.scalar.activation(
                            t1[p0 : p0 + cs, lo:hi],
                            extra[p0 : p0 + cs, lo:hi],
                            ACTF.Copy,
                            bias=0.0,
                            scale=ss_f,
                        )
                        nc.vector.scalar_tensor_tensor(
                            t1[p0 : p0 + cs, lo:hi],
                            ps[p0 : p0 + cs, lo:hi],
                            ss_f,
                            t1[p0 : p0 + cs, lo:hi],
                            OP.mult,
                            OP.add,
                        )
                        nc.scalar.activation(
                            t1[p0 : p0 + cs, lo:hi],
                            t1[p0 : p0 + cs, lo:hi],
                            ACTF.Copy,
                            bias=MAGIC,
                            scale=1.0,
                        )
                    else:
                        nc.scalar.activation(
                            t1[p0 : p0 + cs, lo:hi],
                            ps[p0 : p0 + cs, lo:hi],
                            ACTF.Copy,
                            bias=MAGIC,
                            scale=ss_f,
                        )
                    nc.vector.tensor_scalar(
                        t1[p0 : p0 + cs, lo:hi],
                        t1[p0 : p0 + cs, lo:hi],
                        MAGIC,
                        QMAX,
                        OP.subtract,
                        OP.min,
                    )
                    nc.vector.tensor_scalar(
                        t2[p0 : p0 + cs, lo:hi],
                        t1[p0 : p0 + cs, lo:hi],
                        -QMAX,
                        bt[p0 : p0 + cs, bcol : bcol + 1],
                        OP.max,
                        OP.add,
                    )
                    nc.vector.tensor_scalar(
                        t2[p0 : p0 + cs, lo:hi], t2[p0 : p0 + cs, lo:hi],
                        QMAX, -QMAX, OP.min, OP.max,
                    )
                    nc.sync.dma_start(
                        y_d[
                            i, co0 : co0 + cs,
                            r0 * W + lo : r0 * W + hi,
                        ],
                        t2[p0 : p0 + cs, lo:hi],
                    )

            n_pairs = H // (2 * ROWS_PER_TILE)
            pair_list = [(i, pt) for i in range(IMGS_PER_CORE)
                         for pt in range(n_pairs)]
            for idx, (i, pt) in enumerate(pair_list):
                # interleave image-1 prep between image-0 pairs so per-engine
                # program order matches consumption order
                if idx == 1:
                    make_xq(1)
                    emit_x_slab(1, *SLABS_I1[0], dma_eng=nc.scalar)
                elif idx in (2, 3, 4):
                    emit_x_slab(1, *SLABS_I1[idx - 1], dma_eng=nc.scalar)

                rA = (2 * pt) * ROWS_PER_TILE
                rB = (2 * pt + 1) * ROWS_PER_TILE
                psA = {}
                psB = {}
                for cot in range(3):
                    psA[cot] = psum.tile(
                        [128, ROWS_PER_TILE * W], F32, tag="ps", name="psA"
                    )
                    psB[cot] = psum.tile(
                        [128, ROWS_PER_TILE * W], F32, tag="ps", name="psB"
                    )
                psC = None
                if QUAD4:
                    psC = [
                        psum.tile([128, ROWS_PER_TILE * W], F32, tag="ps",
                                  name="psC"),
                        psum.tile([128, ROWS_PER_TILE * W], F32, tag="ps",
                                  name="psC"),
                    ]

                def rhs(c, r0, kh, kw, lo=0, hi=128):
                    return xq[(i, c)][
                        lo:hi, r0 + kh : r0 + kh + ROWS_PER_TILE, kw : kw + W
                    ]

                def wcol(c, cot, k, lo=0, hi=128):
                    _, cs = CHUNKS[cot]
                    base = COT_OFF[cot] + k * cs
                    return wq[c][lo:hi, base : base + cs]

                # full 128-deep cin chunks for the two 128-wide cout chunks;
                # A row-tile first (only needs slabs < the pair boundary row)
                for ps_, r0 in ((psA, rA), (psB, rB)):
                    for cot in (0, 1):
                        for c in (0, 1):
                            for k in range(K * K):
                                kh, kw = divmod(k, K)
                                nc.tensor.matmul(
                                    ps_[cot][:128, :],
                                    wcol(c, cot, k),
                                    rhs(c, r0, kh, kw),
                                    start=(c == 0 and k == 0),
                                    stop=False,
                                )
                # 64-deep cin remainder: row-packed A/B co-issued pairs
                for cot in (0, 1):
                    co0, cs = CHUNKS[cot]
                    for k in range(K * K):
                        kh, kw = divmod(k, K)
                        last = k == K * K - 1
                        nc.tensor.matmul(
                            psA[cot][:128, :],
                            wcol(2, cot, k, 0, 64),
                            rhs(2, rA, kh, kw, 0, 64),
                            start=False,
                            stop=last,
                        )
                        nc.tensor.matmul(
                            psB[cot][:128, :],
                            wcol(2, cot, k, 64, 128),
                            rhs(2, rB, kh, kw, 64, 128),
                            start=False,
                            stop=last,
                        )
                    emit_epilogue(psA[cot], 0, cot, co0, cs, i, rA)
                    emit_epilogue(psB[cot], 0, cot, co0, cs, i, rB)

                # 64-wide cout remainder: column-pack A/B into the two
                # column halves of the array
                co0, cs = CHUNKS[2]
                for c in (0, 1):
                    for k in range(K * K):
                        kh, kw = divmod(k, K)
                        first = c == 0 and k == 0
                        nc.tensor.matmul(
                            psA[2][0:cs, :],
                            wcol(c, 2, k),
                            rhs(c, rA, kh, kw),
                            start=first,
                            stop=False,
                            tile_position=(0, 0),
                        )
                        nc.tensor.matmul(
                            psB[2][64 : 64 + cs, :],
                            wcol(c, 2, k),
                            rhs(c, rB, kh, kw),
                            start=first,
                            stop=False,
                            tile_position=(0, 64),
                        )
                # cin+cout remainder: 4-way quadrant-packed, two k taps per
                # slot (the lo/hi partition copies of x-chunk-2/w-chunk-2
                # hold identical full-image data, so either copy can feed
                # either row tile; both quadrants in a PE column band
                # accumulate into the same PSUM range)
                # 4-way: odd-k taps go to two spare PSUM banks via the two
                # remaining quadrants (same column band + different bank is
                # the proven-co-issuing pattern); summed in the epilogue.
                kstep = 2 if QUAD4 else 1
                for k in range(0, K * K, kstep):
                    kh, kw = divmod(k, K)
                    last = k == K * K - 1
                    nc.tensor.matmul(
                        psA[2][0:cs, :],
                        wcol(2, 2, k, 0, 64),
                        rhs(2, rA, kh, kw, 0, 64),
                        start=False,
                        stop=last,
                        tile_position=(0, 0),
                    )
                    nc.tensor.matmul(
                        psB[2][64 : 64 + cs, :],
                        wcol(2, 2, k, 0, 64),
                        rhs(2, rB, kh, kw, 0, 64),
                        start=False,
                        stop=last,
                        tile_position=(0, 64),
                    )
                    if QUAD4 and k + 1 < K * K:
                        kh, kw = divmod(k + 1, K)
                        nc.tensor.matmul(
                            psC[0][0:cs, :],
                            wcol(2, 2, k + 1, 64, 128),
                            rhs(2, rA, kh, kw, 64, 128),
                            start=(k == 0),
                            stop=(k + 1 == K * K - 2),
                            tile_position=(64, 0),
                        )
                        nc.tensor.matmul(
                            psC[1][64 : 64 + cs, :],
                            wcol(2, 2, k + 1, 64, 128),
                            rhs(2, rB, kh, kw, 64, 128),
                            start=(k == 0),
                            stop=(k + 1 == K * K - 2),
                            tile_position=(64, 64),
                        )
                nsplit = 2 if idx == len(pair_list) - 1 else 1
                exA = psC[0] if QUAD4 else None
                exB = psC[1] if QUAD4 else None
                emit_epilogue(psA[2], 0, 2, co0, cs, i, rA, nsplit, exA)
                emit_epilogue(psB[2], 64, 2, co0, cs, i, rB, nsplit, exB)

    nc.compile()
    return nc


_BUILD_CACHE = {}


def _get_nc(sx, sw, sb, ss):
    key = (sx, sw, sb, ss)
    if key not in _BUILD_CACHE:
        _BUILD_CACHE[key] = _build(sx, sw, sb, ss)
    return _BUILD_CACHE[key]


def _run(x, weight, bias, step_x, step_w, step_b, shift_scale, trace=False):
    _install_axon_ntff_hook()
    x = np.ascontiguousarray(np.asarray(x, dtype=np.float32))
    w = np.asarray(weight, dtype=np.float32)
    b = np.ascontiguousarray(np.asarray(bias, dtype=np.float32))
    sx = float(np.asarray(step_x))
    sw = float(np.asarray(step_w))
    sb = float(np.asarray(step_b))
    ss = float(np.asarray(shift_scale))

    nc = _get_nc(sx, sw, sb, ss)

    w_t = prep_weight(w)
    x_sh = x.reshape(N_CORES, IMGS_PER_CORE, CIN, HW)

    b_i8 = bias_int8(b, sb, ss, sx, sw)
    in_maps = [
        {"x": x_sh[core], "w": w_t, "b": b_i8} for core in range(N_CORES)
    ]
    res = run_bass_kernel_spmd(
        nc, in_maps, core_ids=list(range(N_CORES)), trace=trace
    )
    out = np.concatenate(
        [res.results[core]["y"].reshape(IMGS_PER_CORE, COUT, H, W)
         for core in range(N_CORES)],
        axis=0,
    )
    return out, res


def kernel(x, weight, bias, step_x, step_w, step_b, shift_scale):
    out, _ = _run(x, weight, bias, step_x, step_w, step_b, shift_scale)
    return out


def kernel_profiled(x, weight, bias, step_x, step_w, step_b, shift_scale):
    return _run(x, weight, bias, step_x, step_w, step_b, shift_scale, trace=True)
